# revision 7
# baseline (speedup 1.0000x reference)
"""Trainium2 Bass kernel for nn_AttModelPerParts (sparse attention + GCN).

Sharding: conv Q/K stacks sharded as 24 units = (part, 256-out-channel
quarter) -> 3 units/core over 8 cores, full batch; one 24KB AllReduce of
partial attention scores; GCN batch-sharded (8 batch elements/core).

Self-contained: builds the Bass/Tile program on first call, runs SPMD on
8 NeuronCores via run_bass_kernel_spmd, gathers full output on host.
"""
import sys
if '/opt/trn_rl_repo' not in sys.path:
    sys.path.insert(0, '/opt/trn_rl_repo')

import math
import numpy as np

import concourse.bass as bass
import concourse.tile as tile
import concourse.mybir as mybir
from concourse import bacc
from concourse.bass_utils import run_bass_kernel_spmd
from concourse.masks import make_identity

F32 = mybir.dt.float32
F32R = mybir.dt.float32r
AF = mybir.ActivationFunctionType
ALU = mybir.AluOpType

# model constants
BS, INPUT_N, FEAT, NP_, PW = 64, 50, 66, 6, 11
D, DCT_N, KS, OUT_N, NS = 1024, 20, 10, 25, 2
VL, VN = 35, 16           # window length / num windows
NCORES, BPC = 8, 8        # cores, batch per core
NQ = 4                    # quarters per part (256-ch units)
BN_EPS = 1e-5

DEBUG = False       # test.py may set kernel.DEBUG = True before first build
SIM_NOCC = False    # replace AllReduce with a local copy (CoreSim debugging)

_CACHE = {}


# ---------------------------------------------------------------- host math
def _get_dct(n):
    i = np.arange(n)
    k = np.arange(n)[:, None]
    w = np.where(k == 0, np.sqrt(1.0 / n), np.sqrt(2.0 / n))
    dct = w * np.cos(np.pi * (i + 0.5) * k / n)
    idct = np.linalg.inv(dct)
    return dct.astype(np.float32), idct.astype(np.float32)


def _core_units(c):
    """Units 3c..3c+2 as (part, quarter), ordered [single, pair, pair]."""
    raw = [(u // NQ, u % NQ) for u in range(3 * c, 3 * c + 3)]
    parts = [p for p, _ in raw]
    if parts[0] == parts[1] == parts[2]:
        ordered = raw
    else:
        from collections import Counter
        cnt = Counter(parts)
        single = [pq for pq in raw if cnt[pq[0]] == 1]
        pair = [pq for pq in raw if cnt[pq[0]] == 2]
        ordered = single + pair
    slot_parts = (ordered[0][0], ordered[1][0])
    return ordered, slot_parts


# ---------------------------------------------------------------- program
def _build_program():
    nc = bacc.Bacc("TRN2", target_bir_lowering=False, debug=False)

    def din(name, shape, dt=F32R):
        return nc.dram_tensor(name, list(shape), dt, kind="ExternalInput")

    # per-core inputs (f32r so sync-DMA into f32r tiles needs no cast)
    x1k_d = din("x1k", [2, 66, 1280])
    x1q_d = din("x1q", [2, 66, 320])
    w1k_d = din("w1k", [2, 66, 1024])
    w1q_d = din("w1q", [2, 66, 1024])
    wk2_d = din("wk2", [3, 40, 128, 256])
    wq2_d = din("wq2", [3, 40, 128, 256])
    assign_d = din("assign", [2, 6])
    srctf_d = din("srctf", [50, 528])
    srcq11_d = din("srcq11", [11, 528])
    m1r_d = din("m1r", [10, 20])
    dctr_d = din("dctr", [35, 20])
    mdir_d = din("mdir", [11, 35])
    attg_d = din("attg", [66, 6, 66])
    w1_d = din("w1", [40, 1024])
    wl_d = din("wl", [4, 1024, 1024])
    w7i_d = din("w7i", [1024, 36])
    gsb_d = din("gsb", [5, 66, 1024], F32)
    bsb_d = din("bsb", [5, 66, 1024], F32)

    out_d = nc.dram_tensor("out", [8, 35, 66], F32, kind="ExternalOutput")
    dbg = {}
    if DEBUG:
        dbg["scloc"] = nc.dram_tensor("dbg_scloc", [6, 1024], F32, kind="ExternalOutput")
        dbg["score"] = nc.dram_tensor("dbg_score", [8, 96], F32, kind="ExternalOutput")
        dbg["att84"] = nc.dram_tensor("dbg_att84", [8, 504], F32, kind="ExternalOutput")
        dbg["sa"] = nc.dram_tensor("dbg_sa", [35, 528], F32, kind="ExternalOutput")
        dbg["x0"] = nc.dram_tensor("dbg_x0", [8, 66, 40], F32, kind="ExternalOutput")
        dbg["y0"] = nc.dram_tensor("dbg_y0", [8, 66, 1024], F32, kind="ExternalOutput")
        dbg["yf"] = nc.dram_tensor("dbg_yf", [8, 66, 1024], F32, kind="ExternalOutput")

    # internal dram
    attpad_d = nc.dram_tensor("attpad", [8 * 504], F32)
    cc_in = nc.dram_tensor("cc_in", [6 * 1024], F32)
    cc_out = nc.dram_tensor("cc_out", [6 * 1024], F32)

    # drain-engine rotation (PSUM readers: ACT + DVE only)
    _rot = [0]

    def drain_copy(out_ap, in_ap):
        _rot[0] ^= 1
        if _rot[0]:
            nc.scalar.copy(out_ap, in_ap)
        else:
            nc.vector.tensor_copy(out_ap, in_ap)

    def drain_relu(out_ap, in_ap):
        _rot[0] ^= 1
        if _rot[0]:
            nc.scalar.activation(out=out_ap, in_=in_ap, func=AF.Relu)
        else:
            nc.vector.tensor_relu(out_ap, in_ap)

    with tile.TileContext(nc) as tc:
        from contextlib import ExitStack
        with ExitStack() as root:
            cst = root.enter_context(tc.tile_pool(name="cst", bufs=1))
            sa_pool = root.enter_context(tc.tile_pool(name="sa_pool", bufs=1))

            # ---- persistent small constants
            srctf = cst.tile([50, 536], F32R)
            srctf_pad = cst.tile([50, 8], F32)
            nc.vector.memset(srctf_pad[:], 0.0)
            nc.vector.tensor_copy(srctf[:, 528:536], srctf_pad[:])
            nc.sync.dma_start(out=srctf[:, 0:528], in_=srctf_d[:])
            srcq11 = cst.tile([11, 528], F32R)
            nc.sync.dma_start(out=srcq11[:], in_=srcq11_d[:])
            m1r = cst.tile([10, 20], F32R)
            nc.sync.dma_start(out=m1r[:], in_=m1r_d[:])
            dctr = cst.tile([35, 20], F32R)
            nc.sync.dma_start(out=dctr[:], in_=dctr_d[:])
            mdir = cst.tile([11, 35], F32R)
            nc.sync.dma_start(out=mdir[:], in_=mdir_d[:])
            attg = cst.tile([66, 6, 66], F32R)
            nc.sync.dma_start(out=attg[:], in_=attg_d[:])
            assign_t = [cst.tile([1, 6], F32R, tag=f"assign{i}", name=f"assign{i}")
                        for i in range(2)]
            nc.sync.dma_start(out=assign_t[0][:], in_=assign_d[0:1, :])
            nc.sync.dma_start(out=assign_t[1][:], in_=assign_d[1:2, :])
            w7i = cst.tile([128, 8, 36], F32R)
            nc.sync.dma_start(
                out=w7i[:],
                in_=bass.AP(tensor=w7i_d.ap().tensor, offset=0,
                            ap=[[36, 128], [128 * 36, 8], [1, 36]]))
            w1t = cst.tile([40, 1024], F32R)
            nc.sync.dma_start(out=w1t[:], in_=w1_d[:])
            identf = cst.tile([128, 128], F32)
            make_identity(nc, identf[:])
            ident = cst.tile([128, 128], F32R)
            nc.vector.tensor_copy(ident[:], identf[:])
            ones_f = cst.tile([128, 1], F32)
            nc.vector.memset(ones_f[:], 1.0)
            ones_r = cst.tile([128, 1], F32R)
            nc.vector.tensor_copy(ones_r[:], ones_f[:])

            score_slots = [cst.tile([1, 1024], F32R, tag=f"scoreslot{i}",
                                    name=f"scoreslot{i}") for i in range(2)]
            score_local = cst.tile([6, 1024], F32)

            # ================= CONV PHASE =================
            with ExitStack() as convs:
                cin = convs.enter_context(tc.tile_pool(name="cin", bufs=1))
                a1p = convs.enter_context(tc.tile_pool(name="a1p", bufs=1))
                wks = convs.enter_context(tc.tile_pool(name="wks", bufs=2))
                wqs = convs.enter_context(tc.tile_pool(name="wqs", bufs=2))
                kvp = convs.enter_context(tc.tile_pool(name="kvp", bufs=2))
                prp = convs.enter_context(tc.tile_pool(name="prp", bufs=1))
                qvp = convs.enter_context(tc.tile_pool(name="qvp", bufs=2))
                # PSUM pools: k2 3 banks + cv1-group 2 + q2 1 + sps/scp 2 = 8
                kpsp = convs.enter_context(tc.tile_pool(name="kpsp", bufs=3, space="PSUM"))
                cv1p = convs.enter_context(tc.tile_pool(name="cv1p", bufs=2, space="PSUM"))
                q2p = convs.enter_context(tc.tile_pool(name="q2p", bufs=1, space="PSUM"))
                scps = convs.enter_context(tc.tile_pool(name="scps", bufs=1, space="PSUM"))

                x1k = cin.tile([66, 2, 1280], F32R)
                nc.sync.dma_start(out=x1k[:], in_=bass.AP(
                    tensor=x1k_d.ap().tensor, offset=0,
                    ap=[[1280, 66], [66 * 1280, 2], [1, 1280]]))
                x1q = cin.tile([66, 2, 320], F32R)
                nc.sync.dma_start(out=x1q[:], in_=bass.AP(
                    tensor=x1q_d.ap().tensor, offset=0,
                    ap=[[320, 66], [66 * 320, 2], [1, 320]]))
                w1k = cin.tile([66, 2, 1024], F32R)
                nc.sync.dma_start(out=w1k[:], in_=bass.AP(
                    tensor=w1k_d.ap().tensor, offset=0,
                    ap=[[1024, 66], [66 * 1024, 2], [1, 1024]]))
                w1q = cin.tile([66, 2, 1024], F32R)
                nc.sync.dma_start(out=w1q[:], in_=bass.AP(
                    tensor=w1q_d.ap().tensor, offset=0,
                    ap=[[1024, 66], [66 * 1024, 2], [1, 1024]]))

                a1k = a1p.tile([128, 8, 64, 20], F32R, tag="a1k")
                a1q = a1p.tile([128, 8, 64, 5], F32R, tag="a1q")

                unit_of_slot = {0: [0], 1: [1, 2]}
                for slot in (0, 1):
                    # conv1K: out = W1K.T @ X1K  -> a1k [ci, ct, b, t]
                    for mt in range(8):
                        for c0, cw in ((0, 512), (512, 512), (1024, 256)):
                            p = cv1p.tile([128, 512], F32, tag="cv1")
                            nc.tensor.matmul(
                                p[:, 0:cw],
                                w1k[:, slot, mt * 128:(mt + 1) * 128],
                                x1k[:, slot, c0:c0 + cw],
                                start=True, stop=True)
                            drain_relu(
                                bass.AP(tensor=a1k.tensor,
                                        offset=a1k.offset + mt * 1280 + c0,
                                        ap=[a1k.ap[0], [1, cw]]),
                                p[:, 0:cw])
                    # conv1Q
                    for mt in range(8):
                        p = cv1p.tile([128, 512], F32, tag="cv1")
                        nc.tensor.matmul(
                            p[:, 0:320], w1q[:, slot, mt * 128:(mt + 1) * 128],
                            x1q[:, slot, :], start=True, stop=True)
                        drain_relu(a1q[:, mt, :, :], p[:, 0:320])

                    for iu, u in enumerate(unit_of_slot[slot]):
                        keyv = kvp.tile([128, 2, 64, 16], F32R, tag="keyv")
                        qps = q2p.tile([64, 256], F32, tag="q2")
                        for mt2 in range(2):
                            kps = [kpsp.tile([128, 512], F32, tag="k2", name=f"kps{_i}")
                                   for _i in range(2)]
                            for ktg in range(5):
                                wkt = wks.tile([128, 8, 128], F32R, tag="wkt")
                                nc.sync.dma_start(
                                    out=wkt[:],
                                    in_=bass.AP(
                                        tensor=wk2_d.ap().tensor,
                                        offset=(u * 40 + ktg * 8) * 128 * 256 + mt2 * 128,
                                        ap=[[256, 128], [128 * 256, 8], [1, 128]]))
                                if mt2 == 0:
                                    wqt = wqs.tile([128, 8, 256], F32R, tag="wqt")
                                    nc.sync.dma_start(
                                        out=wqt[:],
                                        in_=bass.AP(
                                            tensor=wq2_d.ap().tensor,
                                            offset=(u * 40 + ktg * 8) * 128 * 256,
                                            ap=[[256, 128], [128 * 256, 8], [1, 256]]))
                                for k8 in range(8):
                                    kt = ktg * 8 + k8
                                    ct, kp = divmod(kt, 5)
                                    st, sp = (kt == 0), (kt == 39)
                                    for ccb in range(2):
                                        rhs = bass.AP(
                                            tensor=a1k.tensor,
                                            offset=a1k.offset + ct * 1280 + ccb * 32 * 20 + kp,
                                            ap=[a1k.ap[0], [20, 32], [1, 16]])
                                        nc.tensor.matmul(
                                            kps[ccb][:], wkt[:, k8, :], rhs,
                                            start=st, stop=sp)
                                    if mt2 == 0:
                                        lhs_q = bass.AP(
                                            tensor=a1q.tensor,
                                            offset=a1q.offset + ct * 320 + kp,
                                            ap=[a1q.ap[0], [5, 64]])
                                        nc.tensor.matmul(
                                            qps[:], lhs_q, wqt[:, k8, :],
                                            start=(kt == 0), stop=(kt == 39))
                            for ccb in range(2):
                                drain_relu(keyv[:, mt2, ccb * 32:(ccb + 1) * 32, :],
                                           kps[ccb][:])
                        qvT = qvp.tile([64, 256], F32R, tag="qvT")
                        nc.scalar.activation(out=qvT[:], in_=qps[:], func=AF.Relu)
                        qv = qvp.tile([128, 2, 64], F32R, tag="qv")
                        for s in range(2):
                            tp = cv1p.tile([128, 512], F32R, tag="cv1", name="tpq")
                            nc.tensor.transpose(tp[:, 0:64],
                                                qvT[:, s * 128:(s + 1) * 128],
                                                ident[0:64, 0:64])
                            drain_copy(qv[:, s, :], tp[:, 0:64])
                        # --- score partial
                        prod = prp.tile([128, 2, 64, 16], F32R, tag="prod")
                        qv_bc = bass.AP(tensor=qv.tensor, offset=qv.offset,
                                        ap=[qv.ap[0], [64, 2], [1, 64], [0, 16]])
                        nc.vector.tensor_tensor(out=prod[:], in0=keyv[:], in1=qv_bc,
                                                op=ALU.mult)
                        sps = scps.tile([6, 1024], F32, tag="spsscp")
                        for s in range(2):
                            for hb in range(2):
                                nc.tensor.matmul(
                                    sps[0:1, hb * 512:(hb + 1) * 512],
                                    ones_r[:],
                                    prod[:, s, hb * 32:(hb + 1) * 32, :],
                                    start=(s == 0), stop=(s == 1))
                        if iu == 0:
                            drain_copy(score_slots[slot][:], sps[0:1, :])
                        else:
                            stmp = qvp.tile([1, 1024], F32R, tag="stmp")
                            drain_copy(stmp[:], sps[0:1, :])
                            nc.vector.tensor_add(score_slots[slot][:],
                                                 score_slots[slot][:], stmp[:])

                # assign-scatter to [6, 1024]
                scp = scps.tile([6, 1024], F32, tag="spsscp")
                for hb in range(2):
                    nc.tensor.matmul(scp[:, hb * 512:(hb + 1) * 512], assign_t[0][:],
                                     score_slots[0][:, hb * 512:(hb + 1) * 512],
                                     start=True, stop=False)
                    nc.tensor.matmul(scp[:, hb * 512:(hb + 1) * 512], assign_t[1][:],
                                     score_slots[1][:, hb * 512:(hb + 1) * 512],
                                     start=False, stop=True)
                nc.vector.tensor_copy(score_local[:], scp[:])
                if DEBUG:
                    nc.sync.dma_start(out=dbg["scloc"][:], in_=score_local[:])
                nc.sync.dma_start(
                    out=bass.AP(tensor=cc_in.ap().tensor, offset=0,
                                ap=[[1024, 6], [1, 1024]]),
                    in_=score_local[:])

            if SIM_NOCC:
                nc.sync.dma_start(out=cc_out[:], in_=cc_in[:])
            else:
                nc.gpsimd.collective_compute(
                    "AllReduce", ALU.add, replica_groups=[list(range(NCORES))],
                    ins=[cc_in[:]], outs=[cc_out[:]])

            # ================= ATT / SA / X0 / GCN =================
            score_full = cst.tile([8, 96], F32)
            nc.sync.dma_start(
                out=score_full[:],
                in_=bass.AP(tensor=cc_out.ap().tensor, offset=0,
                            ap=[[16, 8], [1024, 6], [1, 16]]))
            if DEBUG:
                nc.sync.dma_start(out=dbg["score"][:], in_=score_full[:])
            tmp = cst.tile([8, 96], F32)
            nc.vector.tensor_scalar_add(tmp[:], score_full[:], 1e-15)
            sums = cst.tile([8, 6], F32)
            nc.vector.tensor_reduce(
                out=sums[:], in_=tmp[:].rearrange("p (a b) -> p a b", b=16),
                axis=mybir.AxisListType.X, op=ALU.add)
            rec = cst.tile([8, 6], F32)
            nc.vector.reciprocal(rec[:], sums[:])
            att84 = cst.tile([8, 504], F32)
            nc.vector.memset(att84[:], 0.0)
            for p in range(6):
                nc.vector.tensor_scalar(
                    out=att84[:, p * 84 + 34:p * 84 + 50],
                    in0=tmp[:, p * 16:(p + 1) * 16],
                    scalar1=rec[:, p:p + 1], scalar2=None, op0=ALU.mult)
            if DEBUG:
                nc.sync.dma_start(out=dbg["att84"][:], in_=att84[:])
            nc.sync.dma_start(
                out=bass.AP(tensor=attpad_d.ap().tensor, offset=0,
                            ap=[[504, 8], [1, 504]]),
                in_=att84[:])
            trev = sa_pool.tile([50, 8, 6, 35], F32R)
            nc.gpsimd.dma_start(
                out=trev[:],
                in_=bass.AP(tensor=attpad_d.ap().tensor, offset=0,
                            ap=[[1, 50], [504, 8], [84, 6], [1, 35]]))

            with ExitStack() as gstack:
                xp = gstack.enter_context(tc.tile_pool(name="xp", bufs=1))
                axp = gstack.enter_context(tc.tile_pool(name="axp", bufs=1))
                t1p = gstack.enter_context(tc.tile_pool(name="t1p", bufs=2))
                wlp = gstack.enter_context(tc.tile_pool(name="wlp", bufs=4))
                gbp = gstack.enter_context(tc.tile_pool(name="gbp", bufs=1))
                zbp = gstack.enter_context(tc.tile_pool(name="zbp", bufs=1))
                # PSUM: axps 3 + zp 3 + misc 2 = 8 banks
                axps_p = gstack.enter_context(tc.tile_pool(name="axps", bufs=3, space="PSUM"))
                zps_p = gstack.enter_context(tc.tile_pool(name="zpsp", bufs=3, space="PSUM"))
                mps_p = gstack.enter_context(tc.tile_pool(name="mpsp", bufs=2, space="PSUM"))

                # SA: att-correlation of src
                sa_ps = [mps_p.tile([35, 4, 6, 12], F32, tag="misc", name=f"sa_ps{_i}") for _i in range(2)]
                for b in range(8):
                    for p in range(6):
                        nc.tensor.matmul(
                            sa_ps[b // 4][:, b % 4, p, :],
                            trev[:, b, p, :],
                            srctf[:, b * 66 + p * 11:b * 66 + p * 11 + 12],
                            start=True, stop=True)
                sa_sb = sa_pool.tile([35, 528], F32R)
                for h in range(2):
                    src_ap = bass.AP(tensor=sa_ps[h].tensor, offset=sa_ps[h].offset,
                                     ap=[sa_ps[h].ap[0], [72, 4], [12, 6], [1, 11]])
                    drain_copy(sa_sb[:, h * 264:(h + 1) * 264], src_ap)
                if DEBUG:
                    sa_f = sa_pool.tile([35, 528], F32)
                    nc.vector.tensor_copy(sa_f[:], sa_sb[:])
                    nc.sync.dma_start(out=dbg["sa"][:], in_=sa_f[:])

                # X0 per-b in layout B [m=66, c=40]
                X0 = [xp.tile([66, 40], F32R, tag=f"x0_{b}", name=f"x0_{b}") for b in range(8)]
                for b in range(8):
                    p = mps_p.tile([66, 264], F32, tag="misc")
                    nc.tensor.matmul(p[:, 0:20], srcq11[0:10, b * 66:(b + 1) * 66],
                                     m1r[:], start=True, stop=True)
                    nc.tensor.matmul(p[:, 20:40], sa_sb[:, b * 66:(b + 1) * 66],
                                     dctr[:], start=True, stop=True)
                    drain_copy(X0[b][:], p[:, 0:40])
                    if DEBUG:
                        x0f = t1p.tile([66, 40], F32, tag="x0f")
                        nc.vector.tensor_copy(x0f[:], p[:, 0:40])
                        nc.sync.dma_start(out=dbg["x0"][b], in_=x0f[:])

                # ---- GCN ----
                Y = [xp.tile([66, 1024], F32R, tag=f"y_{b}", name=f"y_{b}") for b in range(8)]
                X1 = [xp.tile([66, 1024], F32R, tag=f"x1_{b}", name=f"x1_{b}") for b in range(8)]

                def gcn_layer(l, Xin, Xout, c_in):
                    nkt = (c_in + 127) // 128
                    # att-mm B->A
                    ax = axp.tile([128, 8, 8, 66], F32R, tag="ax")
                    for b in range(8):
                        for ct in range(nkt):
                            cw = min(128, c_in - ct * 128)
                            p = axps_p.tile([128, 66], F32, tag="axps")
                            nc.tensor.matmul(p[0:cw, :],
                                             Xin[b][:, ct * 128:ct * 128 + cw],
                                             attg[:, l, :], start=True, stop=True)
                            drain_copy(ax[0:cw, ct, b, :], p[0:cw, :])
                    # weights (quarter-layer chunks)
                    if l > 0:
                        wt = [wlp.tile([128, 2, 1024], F32R, tag="wl", name=f"wt{_i}")
                              for _i in range(4)]
                        for h in range(4):
                            nc.sync.dma_start(
                                out=wt[h][:],
                                in_=bass.AP(tensor=wl_d.ap().tensor,
                                            offset=(l - 1) * 1024 * 1024 + h * 2 * 128 * 1024,
                                            ap=[[1024, 128], [128 * 1024, 2], [1, 1024]]))
                    gt = gbp.tile([66, 1024], F32, tag="gt")
                    nc.sync.dma_start(out=gt[:], in_=gsb_d[l])
                    bt = gbp.tile([66, 1024], F32, tag="bt")
                    nc.sync.dma_start(out=bt[:], in_=bsb_d[l])
                    # w-mm per-b + BN + tanh
                    for b in range(8):
                        for oc in range(2):
                            zp = zps_p.tile([66, 512], F32, tag="zp")
                            for kt in range(nkt):
                                cw = min(128, c_in - kt * 128)
                                if l == 0:
                                    rhs = w1t[0:cw, oc * 512:(oc + 1) * 512]
                                else:
                                    rhs = wt[kt // 2][:, kt % 2, oc * 512:(oc + 1) * 512]
                                nc.tensor.matmul(zp[:], ax[0:cw, kt, b, :], rhs,
                                                 start=(kt == 0), stop=(kt == nkt - 1))
                            t1 = t1p.tile([66, 512], F32, tag="t1")
                            nc.vector.tensor_mul(t1[:], zp[:], gt[:, oc * 512:(oc + 1) * 512])
                            nc.vector.tensor_add(t1[:], t1[:], bt[:, oc * 512:(oc + 1) * 512])
                            nc.scalar.activation(out=Xout[b][:, oc * 512:(oc + 1) * 512],
                                                 in_=t1[:], func=AF.Tanh)

                gcn_layer(0, X0, Y, 40)
                if DEBUG:
                    for b in range(8):
                        yf = sa_pool.tile([66, 1024], F32, tag="ydbg", name="ydbg")
                        nc.vector.tensor_copy(yf[:], Y[b][:])
                        nc.sync.dma_start(out=dbg["y0"][b], in_=yf[:])
                for s in range(2):
                    gcn_layer(1 + 2 * s, Y, X1, 1024)
                    gcn_layer(2 + 2 * s, X1, X1, 1024)
                    for b in range(8):
                        nc.vector.tensor_add(Y[b][:], X1[b][:], Y[b][:])
                if DEBUG:
                    for b in range(8):
                        yf = sa_pool.tile([66, 1024], F32, tag="ydbg", name="ydbg")
                        nc.vector.tensor_copy(yf[:], Y[b][:])
                        nc.sync.dma_start(out=dbg["yf"][b], in_=yf[:])

                # ---- gc7 + idct folding ----
                yA = axp.tile([128, 8, 8, 66], F32R, tag="ax")
                for b in range(8):
                    for ct in range(8):
                        p = axps_p.tile([128, 66], F32R, tag="axps", name="pty")
                        nc.tensor.transpose(p[0:128, 0:66],
                                            Y[b][:, ct * 128:(ct + 1) * 128],
                                            ident[0:66, 0:66])
                        drain_copy(yA[:, ct, b, :], p[:, 0:66])
                zb = [zbp.tile([66, 35], F32R, tag=f"zb_{b}", name=f"zb_{b}") for b in range(8)]
                for b in range(8):
                    p = zps_p.tile([66, 512], F32, tag="zp")
                    for kt in range(8):
                        nc.tensor.matmul(p[:, 0:36], yA[:, kt, b, :], w7i[:, kt, :],
                                         start=(kt == 0), stop=(kt == 7))
                    drain_copy(zb[b][:], p[:, 0:35])
                fin = [mps_p.tile([35, 264], F32, tag="misc", name=f"fin{_i}") for _i in range(2)]
                for h in range(2):
                    nc.tensor.matmul(fin[h][:], mdir[:],
                                     srcq11[:, h * 264:(h + 1) * 264],
                                     start=True, stop=False)
                for b in range(8):
                    nc.tensor.matmul(
                        fin[b // 4][:, (b % 4) * 66:(b % 4) * 66 + 66],
                        zb[b][:], attg[:, 5, :],
                        start=False, stop=(b % 4 == 3))
                out_sb = sa_pool.tile([35, 528], F32)
                nc.vector.tensor_copy(out_sb[:, 0:264], fin[0][:])
                nc.vector.tensor_copy(out_sb[:, 264:528], fin[1][:])
                nc.sync.dma_start(
                    out=bass.AP(tensor=out_d.ap().tensor, offset=0,
                                ap=[[66, 35], [2310, 8], [1, 66]]),
                    in_=out_sb[:])

    nc.compile()
    return nc


# ---------------------------------------------------------------- host prep
def _host_prep(inputs):
    src = np.asarray(inputs["src"], np.float32)          # [64, 50, 66]
    dct_m_f, idct_f = _get_dct(VL)
    dct_m = dct_m_f[:DCT_N]                              # [20, 35]
    idct_m = idct_f[:, :DCT_N]                           # [35, 20]

    # dct_in combined-frames matrix  [20, 10]
    M1 = dct_m[:, :10].copy()
    M1[:, 9] += dct_m[:, 10:].sum(axis=1)
    m1r = np.ascontiguousarray(M1.T)                     # [10, 20]
    dctr = np.ascontiguousarray(dct_m[:, ::-1].T)        # [35, 20]
    M_direct = idct_m @ M1                               # [35, 10]
    b7i = idct_m @ np.asarray(inputs["gc7_b"], np.float32)[:DCT_N]  # [35]
    mdir = np.concatenate([M_direct.T, b7i[None, :]], 0).astype(np.float32)  # [11, 35]

    # conv1 im2col of src windows (full batch)
    s = src / 1000.0
    wk = np.lib.stride_tricks.sliding_window_view(s[:, :25, :], 6, axis=1)  # [64,20,66,6]
    arrk = wk.transpose(2, 3, 0, 1)                      # [66(f), 6(k), 64, 20]
    wq = np.lib.stride_tricks.sliding_window_view(s[:, 40:50, :], 6, axis=1)  # [64,5,66,6]
    arrq = wq.transpose(2, 3, 0, 1)                      # [66, 6, 64, 5]

    cK1 = np.asarray(inputs["convK_w1"], np.float32)     # [6,1024,11,6]
    cQ1 = np.asarray(inputs["convQ_w1"], np.float32)
    cK2 = np.asarray(inputs["convK_w2"], np.float32)     # [6,1024,1024,5]
    cQ2 = np.asarray(inputs["convQ_w2"], np.float32)

    # gcn consts
    c = np.float32(1.0 / math.sqrt(1.0 + BN_EPS))

    def gb(g, be, bgc):
        G = g.reshape(66, 1024) * c
        B = bgc[None, :] * G + be.reshape(66, 1024)
        return G.astype(np.float32), B.astype(np.float32)

    gs, bs_ = [], []
    G0, B0 = gb(np.asarray(inputs["bn1_g"], np.float32),
                np.asarray(inputs["bn1_b"], np.float32),
                np.asarray(inputs["gc1_b"], np.float32))
    gs.append(G0); bs_.append(B0)
    for st in range(2):
        G, Bv = gb(np.asarray(inputs["blk_g1"], np.float32)[st],
                   np.asarray(inputs["blk_be1"], np.float32)[st],
                   np.asarray(inputs["blk_b1"], np.float32)[st])
        gs.append(G); bs_.append(Bv)
        G, Bv = gb(np.asarray(inputs["blk_g2"], np.float32)[st],
                   np.asarray(inputs["blk_be2"], np.float32)[st],
                   np.asarray(inputs["blk_b2"], np.float32)[st])
        gs.append(G); bs_.append(Bv)
    gsb = np.stack(gs)      # [5, 66, 1024]  (gc1, s0l1, s0l2, s1l1, s1l2)
    bsb = np.stack(bs_)

    attg = np.stack([
        np.asarray(inputs["gc1_att"], np.float32).T,
        np.asarray(inputs["blk_att1"], np.float32)[0].T,
        np.asarray(inputs["blk_att2"], np.float32)[0].T,
        np.asarray(inputs["blk_att1"], np.float32)[1].T,
        np.asarray(inputs["blk_att2"], np.float32)[1].T,
        np.asarray(inputs["gc7_att"], np.float32).T,
    ])                                                   # [6, 66, 66]
    attg = np.ascontiguousarray(attg.transpose(1, 0, 2))  # [66, 6, 66]
    w1 = np.asarray(inputs["gc1_w"], np.float32)         # [40, 1024]
    wl = np.stack([
        np.asarray(inputs["blk_w1"], np.float32)[0],
        np.asarray(inputs["blk_w2"], np.float32)[0],
        np.asarray(inputs["blk_w1"], np.float32)[1],
        np.asarray(inputs["blk_w2"], np.float32)[1],
    ])                                                   # [4, 1024, 1024]
    w7i = (np.asarray(inputs["gc7_w"], np.float32)[:, :DCT_N] @ idct_m.T).astype(np.float32)
    w7i = np.concatenate([w7i, np.zeros((1024, 1), np.float32)], axis=1)  # pad N to 36

    in_maps = []
    for core in range(NCORES):
        ordered, (pA, pB) = _core_units(core)
        x1k = np.stack([arrk[11 * p:11 * p + 11].reshape(66, 1280) for p in (pA, pB)])
        x1q = np.stack([arrq[11 * p:11 * p + 11].reshape(66, 320) for p in (pA, pB)])
        w1k_ = np.stack([cK1[p].reshape(1024, 66).T for p in (pA, pB)])
        w1q_ = np.stack([cQ1[p].reshape(1024, 66).T for p in (pA, pB)])

        def wslices(carr, pq):
            p, q = pq
            out = np.empty((40, 128, 256), np.float32)
            for ct in range(8):
                for kp in range(5):
                    out[ct * 5 + kp] = carr[p, 256 * q:256 * q + 256,
                                            128 * ct:128 * ct + 128, kp].T
            return out
        wk2 = np.stack([wslices(cK2, pq) for pq in ordered])
        wq2 = np.stack([wslices(cQ2, pq) for pq in ordered])

        assign = np.zeros((2, 6), np.float32)
        assign[0, pA] = 1.0
        assign[1, pB] = 1.0

        bsl = src[8 * core:8 * core + 8]                  # [8, 50, 66]
        srctf = np.ascontiguousarray(bsl.transpose(1, 0, 2).reshape(50, 528))
        srcq11 = np.concatenate([srctf[40:50], np.ones((1, 528), np.float32)], 0)

        in_maps.append({
            "x1k": np.ascontiguousarray(x1k), "x1q": np.ascontiguousarray(x1q),
            "w1k": np.ascontiguousarray(w1k_), "w1q": np.ascontiguousarray(w1q_),
            "wk2": np.ascontiguousarray(wk2), "wq2": np.ascontiguousarray(wq2),
            "assign": assign, "srctf": srctf, "srcq11": srcq11,
            "m1r": m1r, "dctr": dctr, "mdir": mdir, "attg": attg,
            "w1": w1, "wl": wl, "w7i": w7i, "gsb": gsb, "bsb": bsb,
        })
    return in_maps


def kernel(**inputs):
    if "nc" not in _CACHE:
        _CACHE["nc"] = _build_program()
    nc = _CACHE["nc"]
    in_maps = _host_prep(inputs)
    res = run_bass_kernel_spmd(nc, in_maps, core_ids=list(range(NCORES)))
    _CACHE["last_results"] = res
    outs = [res.results[c]["out"] for c in range(NCORES)]       # each [8, 35, 66]
    full = np.concatenate(outs, axis=0)                         # [64, 35, 66]
    return np.ascontiguousarray(full[:, :, None, :]).astype(np.float32)
